# revision 1
# baseline (speedup 1.0000x reference)
"""LeNet-style ClientNet (dense_cnn) on 8 Trainium2 NeuronCores.

Strategy (data-parallel, batch sharded 8x1024):
  host: ps-weighted average of the 16 client stacks (tiny einsum), weights
        pre-shaped into banded lhsT layouts for the PE.
  core: conv1 as one K=51 matmul per N-block (50 = 10 relrow x 5 dx banded
        rows + ones row carrying the bias), relu+maxpool fused on DVE/GPSIMD,
        conv2 as 5 dx-accumulated K=120 matmuls (20 cin x 6 relrow), fc1 as
        16 accumulated K=50 matmuls (one per spatial tap), fc2 K=125 x4.
        All matmuls float32r (full PE rate, ~tf32 accuracy), psum fp32.
"""

import contextlib
import sys

import numpy as np

sys.path.insert(0, "/opt/trn_rl_repo")

import concourse.bass as bass  # noqa: E402
import concourse.bacc as bacc  # noqa: E402
import concourse.mybir as mybir  # noqa: E402
from concourse.tile import TileContext  # noqa: E402

F32R = mybir.dt.float32r
F32 = mybir.dt.float32
MAX = mybir.AluOpType.max
ADD = mybir.AluOpType.add

NCORES = 8
BC = 1024            # samples per core
CH = 32              # samples per chunk
NCH = BC // CH       # 32 chunks
QC = 8               # chunks per fc group (256 samples)
NQ = NCH // QC       # 4 fc groups


def _ap(t, off, dims):
    return bass.AP(tensor=t.tensor, offset=t.offset + off, ap=[list(d) for d in dims])


def _pitch(t):
    return t.ap[0][0]


def build_host_weights(ps, conv1_w, conv1_b, conv2_w, conv2_b,
                       fc1_w, fc1_b, fc2_w, fc2_b):
    ps = np.asarray(ps, np.float64)
    W1 = np.einsum("n,noihw->oihw", ps, np.asarray(conv1_w, np.float64))[:, 0]  # [20,5,5]
    b1 = ps @ np.asarray(conv1_b, np.float64)                                   # [20]
    W2 = np.einsum("n,noihw->oihw", ps, np.asarray(conv2_w, np.float64))        # [50,20,5,5]
    b2 = ps @ np.asarray(conv2_b, np.float64)                                   # [50]
    Wf1 = np.einsum("n,nof->of", ps, np.asarray(fc1_w, np.float64))             # [500,800]
    bf1 = ps @ np.asarray(fc1_b, np.float64)                                    # [500]
    Wf2 = np.einsum("n,nof->of", ps, np.asarray(fc2_w, np.float64))             # [10,500]
    bf2 = ps @ np.asarray(fc2_b, np.float64)                                    # [10]

    # conv1 lhsT [51, 120]: k = dx*10 + relrow (rows 0..49), row 50 = bias.
    # m = e*60 + o*3 + t ; output row y = 6g + 2t + e ; input row 6g + relrow,
    # dy = relrow - (2t + e) in 0..4.
    # conv1 lhsT [41, 104]: k = dx*8 + rr (rows 0..39), row 40 = bias ones-row.
    # m = e*64 + u*20 + o ; out row y = 4G + 2u + e ; input row 4G + rr,
    # dy = rr - (2u + e) in 0..4.
    L1 = np.zeros((41, 104), np.float32)
    for dx in range(5):
        for rr in range(8):
            for e in range(2):
                for u in range(2):
                    for o in range(20):
                        dy = rr - (2 * u + e)
                        if 0 <= dy <= 4:
                            L1[dx * 8 + rr, e * 64 + u * 20 + o] = W1[o, dy, dx]
    for e in range(2):
        for u in range(2):
            for o in range(20):
                L1[40, e * 64 + u * 20 + o] = b1[o]

    # conv2 lhsT [120, 5*100]: k = c*6 + relrow, m(dx) = dx*100 + e*50 + o.
    # out row y' = 2gg + e ; pooled input row 2gg + relrow ; dy = relrow - e.
    L2 = np.zeros((121, 570), np.float32)
    for dx in range(5):
        for c in range(20):
            for rr in range(6):
                for e in range(2):
                    dy = rr - e
                    if 0 <= dy <= 4:
                        L2[rr * 20 + c, dx * 114 + e * 64:dx * 114 + e * 64 + 50] = \
                            W2[:, c, dy, dx]
    for e in range(2):
        L2[120, e * 64:e * 64 + 50] = b2

    # fc1 lhsT [50, 16*500]: tap f = gg*4 + xp; torch feature id = o*16 + f.
    LF1 = np.zeros((51, 16 * 500), np.float32)
    for gg in range(4):
        for xp in range(4):
            f = gg * 4 + xp
            for o in range(50):
                LF1[o, f * 500:(f + 1) * 500] = Wf1[:, o * 16 + f]
    LF1[50, 0:500] = bf1

    # fc2 lhsT [125, 4*10]
    LF2 = np.zeros((126, 40), np.float32)
    for c in range(4):
        LF2[0:125, c * 10:(c + 1) * 10] = Wf2[:, c * 125:(c + 1) * 125].T
    LF2[125, 0:10] = bf2

    return dict(
        lhsT1=L1,
        lhsT2=L2.astype(np.float32),
        lf1=LF1.astype(np.float32),
        lf2=LF2.astype(np.float32),
        onesv=np.ones((4096,), np.float32),
    )


def stage_x(xc):
    """Host-side im2col-lite: [BC,784] -> [NCH, 41, CH*144] staged conv1 rhs."""
    x3 = np.asarray(xc, np.float32).reshape(NCH, CH, 28, 28)
    st = np.empty((NCH, 41, CH, 144), np.float32)
    st[:, 40] = 1.0
    rows_base = 4 * np.arange(6)
    for dx in range(5):
        for rr in range(8):
            k = dx * 8 + rr
            rows = rows_base + rr
            st[:, k] = x3[:, :, rows, :][:, :, :, dx:dx + 24].reshape(NCH, CH, 144)
    return st.reshape(NCH, 41, CH * 144)


def build_nc():
    nc = bacc.Bacc()
    x_d = nc.dram_tensor("x", [NCH, 41, CH * 144], F32R, kind="ExternalInput")
    L1_d = nc.dram_tensor("lhsT1", [41, 104], F32R, kind="ExternalInput")
    L2_d = nc.dram_tensor("lhsT2", [121, 570], F32R, kind="ExternalInput")
    LF1_d = nc.dram_tensor("lf1", [51, 8000], F32R, kind="ExternalInput")
    LF2_d = nc.dram_tensor("lf2", [126, 40], F32R, kind="ExternalInput")
    ON_d = nc.dram_tensor("onesv", [4096], F32R, kind="ExternalInput")
    out_d = nc.dram_tensor("out", [BC, 10], F32, kind="ExternalOutput")

    ctx = contextlib.ExitStack()
    with ctx:
        with TileContext(nc) as tc:
            with contextlib.ExitStack() as pctx:
                cpool = pctx.enter_context(tc.tile_pool(name="const", bufs=1))
                r1p = pctx.enter_context(tc.tile_pool(name="r1", bufs=2))
                p1p = pctx.enter_context(tc.tile_pool(name="p1", bufs=2))
                y1p = pctx.enter_context(tc.tile_pool(name="y1", bufs=2))
                c2rp = pctx.enter_context(tc.tile_pool(name="c2r", bufs=2))
                p2p = pctx.enter_context(tc.tile_pool(name="p2", bufs=2))
                t2p = pctx.enter_context(tc.tile_pool(name="t2", bufs=2))
                y2p = pctx.enter_context(tc.tile_pool(name="y2", bufs=2))
                y3p = pctx.enter_context(tc.tile_pool(name="y3", bufs=2))
                osbp = pctx.enter_context(tc.tile_pool(name="osb", bufs=2))
                e1p = pctx.enter_context(tc.tile_pool(name="e1", bufs=2))
                p1bp = pctx.enter_context(tc.tile_pool(name="p1b", bufs=2))
                p2bp = pctx.enter_context(tc.tile_pool(name="p2b", bufs=2))
                e2p = pctx.enter_context(tc.tile_pool(name="e2", bufs=2))
                ps1p = pctx.enter_context(tc.tile_pool(name="ps1", bufs=2, space="PSUM"))
                ps2p = pctx.enter_context(tc.tile_pool(name="ps2", bufs=2, space="PSUM"))
                ps3p = pctx.enter_context(tc.tile_pool(name="ps3", bufs=2, space="PSUM"))
                ps4p = pctx.enter_context(tc.tile_pool(name="ps4", bufs=2, space="PSUM"))
                # --- constants ---
                L1 = cpool.tile([41, 104], F32R)
                nc.sync.dma_start(out=L1[:, :], in_=L1_d[:, :])
                L2 = cpool.tile([121, 570], F32R)
                nc.sync.dma_start(out=L2[:, :], in_=L2_d[:, :])
                LF1 = cpool.tile([51, 8000], F32R)
                nc.sync.dma_start(out=LF1[:, :], in_=LF1_d[:, :])
                LF2 = cpool.tile([126, 40], F32R)
                nc.sync.dma_start(out=LF2[:, :], in_=LF2_d[:, :])

                y2_cur = None
                c2r_tiles = []
                for j in range(2):
                    t_ = c2rp.tile([121, CH * 48], F32R)
                    nc.sync.dma_start(
                        out=_ap(t_[:, :], 120 * _pitch(t_[:, :]),
                                [[_pitch(t_[:, :]), 1], [1, CH * 48]]),
                        in_=_ap(ON_d[:], 0, [[0, 1], [1, CH * 48]]),
                    )
                    c2r_tiles.append(t_)
                for i in range(NCH):
                    q = i // QC
                    # ---- conv1 rhs: host-staged, one DMA ----
                    R1 = r1p.tile([41, CH * 144], F32R)
                    pr = _pitch(R1[:, :])
                    nc.sync.dma_start(out=R1[:, :], in_=x_d[i, :, :])
                    # ---- conv1 matmuls + evict + pool-x ----
                    P1 = p1p.tile([104, CH * 72], F32R)
                    pp1 = _pitch(P1[:, :])
                    for bs in range(CH // 2):
                        ps1 = ps1p.tile([104, 288], F32)
                        nc.tensor.matmul(
                            ps1[:, :], L1[:, :],
                            _ap(R1[:, :], bs * 288, [[pr, 41], [1, 288]]),
                            start=True, stop=True,
                        )
                        E1 = e1p.tile([104, 288], F32)
                        pe1 = _pitch(E1[:, :])
                        nc.scalar.copy(out=E1[:, :], in_=ps1[:, :])
                        nc.vector.tensor_tensor(
                            out=_ap(P1[:, :], bs * 144,
                                    [[pp1, 104], [72, 2], [12, 6], [1, 12]]),
                            in0=_ap(E1[:, :], 0,
                                    [[pe1, 104], [144, 2], [24, 6], [2, 12]]),
                            in1=_ap(E1[:, :], 1,
                                    [[pe1, 104], [144, 2], [24, 6], [2, 12]]),
                            op=MAX,
                        )
                    # ---- conv1 pool-y + relu ----
                    P1B = p1bp.tile([40, CH * 72], F32R)
                    nc.sync.dma_start(out=P1B[:, :], in_=P1[64:104, :])
                    Y1 = y1p.tile([40, CH * 72], F32R)
                    nc.vector.tensor_tensor(
                        out=Y1[:, :], in0=P1[0:40, :], in1=P1B[:, :], op=MAX)
                    nc.vector.tensor_scalar_max(out=Y1[:, :], in0=Y1[:, :],
                                                scalar1=0.0)
                    # ---- shuffle Y1 -> C2R (6 DMAs) ----
                    C2R = c2r_tiles[i % 2]
                    pc = _pitch(C2R[:, :])
                    py1 = _pitch(Y1[:, :])
                    for u in range(2):
                        for v in range(3):
                            nc.sync.dma_start(
                                out=_ap(C2R[:, :], (2 * v + u) * 20 * pc,
                                        [[pc, 20], [48, CH], [1, 48]]),
                                in_=_ap(Y1[:, :], u * 20 * py1 + v * 12,
                                        [[py1, 20], [72, CH], [1, 48]]),
                            )
                    # ---- conv2: groups of 16 samples ----
                    P2 = p2p.tile([114, CH * 16], F32R)
                    pp2 = _pitch(P2[:, :])
                    for bg in range(CH // 16):
                        ps2 = ps2p.tile([114, 512], F32)
                        pq = _pitch(ps2[:, :])
                        for dx in range(5):
                            nc.tensor.matmul(
                                ps2[:, :],
                                _ap(L2[:, :], dx * 114,
                                    [[_pitch(L2[:, :]), 121], [1, 114]]),
                                _ap(C2R[:, :], bg * 16 * 48 + dx,
                                    [[pc, 121], [48, 16], [12, 4], [1, 8]]),
                                start=(dx == 0), stop=(dx == 4),
                            )
                        E2 = e2p.tile([114, 512], F32)
                        pe2 = _pitch(E2[:, :])
                        nc.scalar.copy(out=E2[:, :], in_=ps2[:, :])
                        nc.vector.tensor_tensor(
                            out=_ap(P2[:, :], bg * 256,
                                    [[pp2, 114], [16, 16], [4, 4], [1, 4]]),
                            in0=_ap(E2[:, :], 0,
                                    [[pe2, 114], [32, 16], [8, 4], [2, 4]]),
                            in1=_ap(E2[:, :], 1,
                                    [[pe2, 114], [32, 16], [8, 4], [2, 4]]),
                            op=MAX,
                        )
                    # ---- conv2 pool-y (gpsimd) + bias/relu into Y2 ----
                    P2B = p2bp.tile([50, CH * 16], F32R)
                    nc.sync.dma_start(out=P2B[:, :], in_=P2[64:114, :])
                    T2 = t2p.tile([50, CH * 16], F32R)
                    nc.vector.tensor_tensor(
                        out=T2[:, :], in0=P2[0:50, :], in1=P2B[:, :], op=MAX)
                    if i % QC == 0:
                        y2_cur = y2p.tile([51, QC * CH * 16], F32R)
                        nc.sync.dma_start(
                            out=_ap(y2_cur[:, :], 50 * _pitch(y2_cur[:, :]),
                                    [[_pitch(y2_cur[:, :]), 1], [1, QC * CH * 16]]),
                            in_=_ap(ON_d[:], 0, [[0, 1], [1, QC * CH * 16]]),
                        )
                    Y2 = y2_cur
                    nc.vector.tensor_scalar_max(
                        out=Y2[0:50, (i % QC) * CH * 16:(i % QC + 1) * CH * 16],
                        in0=T2[:, :], scalar1=0.0,
                    )
                    # ---- fc1 + fc2 per completed 256-sample group ----
                    if i % QC == QC - 1:
                        NB = QC * CH  # 256
                        py2 = _pitch(Y2[:, :])
                        Y3 = y3p.tile([126, 4 * NB], F32R)
                        nc.sync.dma_start(
                            out=_ap(Y3[:, :], 125 * _pitch(Y3[:, :]),
                                    [[_pitch(Y3[:, :]), 1], [1, 4 * NB]]),
                            in_=_ap(ON_d[:], 0, [[0, 1], [1, 4 * NB]]),
                        )
                        for c in range(4):
                            ps3 = ps3p.tile([125, NB], F32)
                            for f in range(16):
                                nc.tensor.matmul(
                                    ps3[:, :],
                                    _ap(LF1[:, :], f * 500 + c * 125,
                                        [[_pitch(LF1[:, :]), 51], [1, 125]]),
                                    _ap(Y2[:, :], f, [[py2, 51], [16, NB]]),
                                    start=(f == 0), stop=(f == 15),
                                )
                            nc.vector.tensor_scalar_max(
                                out=Y3[0:125, c * NB:(c + 1) * NB],
                                in0=ps3[:, :], scalar1=0.0,
                            )
                        ps4 = ps4p.tile([10, NB], F32)
                        for c in range(4):
                            nc.tensor.matmul(
                                ps4[:, :],
                                _ap(LF2[:, :], c * 10,
                                    [[_pitch(LF2[:, :]), 126], [1, 10]]),
                                _ap(Y3[:, :], c * NB,
                                    [[_pitch(Y3[:, :]), 126], [1, NB]]),
                                start=(c == 0), stop=(c == 3),
                            )
                        OUT = osbp.tile([10, NB], F32)
                        nc.vector.tensor_copy(out=OUT[:, :], in_=ps4[:, :])
                        nc.sync.dma_start(
                            out=_ap(out_d[:], q * NB * 10, [[1, 10], [10, NB]]),
                            in_=_ap(OUT[:, :], 0, [[_pitch(OUT[:, :]), 10], [1, NB]]),
                        )
    return nc


_NC_CACHE = None


def kernel(x, ps, conv1_w, conv1_b, conv2_w, conv2_b, fc1_w, fc1_b, fc2_w, fc2_b):
    global _NC_CACHE
    from concourse import bass_utils

    w = build_host_weights(ps, conv1_w, conv1_b, conv2_w, conv2_b,
                           fc1_w, fc1_b, fc2_w, fc2_b)
    if _NC_CACHE is None:
        _NC_CACHE = build_nc()
        _NC_CACHE.finalize()
    nc = _NC_CACHE

    x = np.asarray(x, np.float32).reshape(8192, 784)
    in_maps = []
    for c in range(NCORES):
        m = dict(w)
        m["x"] = stage_x(x[c * BC:(c + 1) * BC])
        in_maps.append(m)
    res = bass_utils.run_bass_kernel_spmd(nc, in_maps, core_ids=list(range(NCORES)))
    out = np.concatenate([r["out"] for r in res.results], axis=0)
    return out.astype(np.float32)



# revision 2
# speedup vs baseline: 5.4013x; 5.4013x over previous
"""LeNet-style ClientNet (dense_cnn) on 8 Trainium2 NeuronCores.

Strategy (data-parallel, batch sharded 8x1024):
  host: ps-weighted average of the 16 client stacks (tiny einsum), weights
        pre-shaped into banded lhsT layouts for the PE; everything packed
        into ONE f32 blob per core (x + weights, bf16 where tolerable) so
        the axon-tunneled PJRT path pays a single per-array transfer.
  core: im2col for conv1 done on-device via strided DMAs from the raw x
        section of the blob (30 DMAs per 32-sample chunk), conv1 as one
        K=41 bf16 matmul per 2-sample block (banded rows + ones row for
        bias), relu+maxpool fused on DVE, conv2 as 5 dx-accumulated K=114
        bf16 matmuls, fc1 as 16 accumulated K=51 bf16 matmuls (one per
        spatial tap), fc2 K=126 x4 in f32r. psum always fp32.
"""

import contextlib
import sys

import numpy as np
import ml_dtypes

sys.path.insert(0, "/opt/trn_rl_repo")

import concourse.bass as bass  # noqa: E402
import concourse.bacc as bacc  # noqa: E402
import concourse.mybir as mybir  # noqa: E402
from concourse.tile import TileContext  # noqa: E402

F32R = mybir.dt.float32r
F32 = mybir.dt.float32
BF16 = mybir.dt.bfloat16
MAX = mybir.AluOpType.max
ADD = mybir.AluOpType.add
BFNP = ml_dtypes.bfloat16

NCORES = 8
BC = 1024            # samples per core
CH = 32              # samples per chunk
NCH = BC // CH       # 32 chunks
QC = 8               # chunks per fc group (256 samples)
NQ = NCH // QC       # 4 fc groups

# ---- single-blob layout (units: f32 elements; bf16 sections hold 2/elem) ----
OFF_X = 0                       # x bf16 [1024,784]
XF = BC * 784 // 2              # 401408
OFF_L1 = OFF_X + XF             # conv1 lhsT bf16 [41,104]
L1F = 41 * 104 // 2             # 2132
OFF_L2 = OFF_L1 + L1F           # conv2 lhsT bf16 [121,570]
L2F = 121 * 570 // 2            # 34485
OFF_LF1 = OFF_L2 + L2F          # fc1 lhsT bf16 [51,8000]
LF1F = 51 * 8000 // 2           # 204000
OFF_LF2 = OFF_LF1 + LF1F        # fc2 lhsT f32 [126,40]
LF2F = 126 * 40                 # 5040
OFF_ON32 = OFF_LF2 + LF2F       # f32 ones [4096]
ON32F = 4096
OFF_ON16 = OFF_ON32 + ON32F     # bf16 ones [8192]
ON16F = 8192 // 2
NTOT = OFF_ON16 + ON16F         # 655257


def _ap(t, off, dims):
    return bass.AP(tensor=t.tensor, offset=t.offset + off, ap=[list(d) for d in dims])


def _pitch(t):
    return t.ap[0][0]


def build_host_weights(ps, conv1_w, conv1_b, conv2_w, conv2_b,
                       fc1_w, fc1_b, fc2_w, fc2_b):
    ps = np.asarray(ps, np.float64)
    W1 = np.einsum("n,noihw->oihw", ps, np.asarray(conv1_w, np.float64))[:, 0]  # [20,5,5]
    b1 = ps @ np.asarray(conv1_b, np.float64)                                   # [20]
    W2 = np.einsum("n,noihw->oihw", ps, np.asarray(conv2_w, np.float64))        # [50,20,5,5]
    b2 = ps @ np.asarray(conv2_b, np.float64)                                   # [50]
    Wf1 = np.einsum("n,nof->of", ps, np.asarray(fc1_w, np.float64))             # [500,800]
    bf1 = ps @ np.asarray(fc1_b, np.float64)                                    # [500]
    Wf2 = np.einsum("n,nof->of", ps, np.asarray(fc2_w, np.float64))             # [10,500]
    bf2 = ps @ np.asarray(fc2_b, np.float64)                                    # [10]

    # conv1 lhsT [41, 104]: k = dx*8 + rr (rows 0..39), row 40 = bias ones-row.
    # m = e*64 + u*20 + o ; out row y = 4G + 2u + e ; input row 4G + rr,
    # dy = rr - (2u + e) in 0..4.
    L1 = np.zeros((41, 104), np.float32)
    for dx in range(5):
        for rr in range(8):
            for e in range(2):
                for u in range(2):
                    for o in range(20):
                        dy = rr - (2 * u + e)
                        if 0 <= dy <= 4:
                            L1[dx * 8 + rr, e * 64 + u * 20 + o] = W1[o, dy, dx]
    for e in range(2):
        for u in range(2):
            for o in range(20):
                L1[40, e * 64 + u * 20 + o] = b1[o]

    # conv2 lhsT [121, 570]: k = rr*20 + c, m(dx) = dx*114 + e*64 + o.
    # out row y' = 2gg + e ; pooled input row 2gg + rr ; dy = rr - e.
    L2 = np.zeros((121, 570), np.float32)
    for dx in range(5):
        for c in range(20):
            for rr in range(6):
                for e in range(2):
                    dy = rr - e
                    if 0 <= dy <= 4:
                        L2[rr * 20 + c, dx * 114 + e * 64:dx * 114 + e * 64 + 50] = \
                            W2[:, c, dy, dx]
    for e in range(2):
        L2[120, e * 64:e * 64 + 50] = b2

    # fc1 lhsT [51, 16*500]: tap f = gg*4 + xp; torch feature id = o*16 + f.
    LF1 = np.zeros((51, 8000), np.float32)
    for gg in range(4):
        for xp in range(4):
            f = gg * 4 + xp
            for o in range(50):
                LF1[o, f * 500:(f + 1) * 500] = Wf1[:, o * 16 + f]
    LF1[50, 0:500] = bf1

    # fc2 lhsT [126, 40]
    LF2 = np.zeros((126, 40), np.float32)
    for c in range(4):
        LF2[0:125, c * 10:(c + 1) * 10] = Wf2[:, c * 125:(c + 1) * 125].T
    LF2[125, 0:10] = bf2

    # ---- pack weight sections of the blob (f32 view) ----
    wsec = np.zeros(NTOT - OFF_L1, np.float32)

    def put16(off_f32, arr):
        v = np.ascontiguousarray(arr.astype(BFNP)).reshape(-1).view(np.float32)
        wsec[off_f32 - OFF_L1:off_f32 - OFF_L1 + v.size] = v

    put16(OFF_L1, L1)
    put16(OFF_L2, L2)
    put16(OFF_LF1, LF1)
    wsec[OFF_LF2 - OFF_L1:OFF_LF2 - OFF_L1 + LF2F] = LF2.reshape(-1)
    wsec[OFF_ON32 - OFF_L1:OFF_ON32 - OFF_L1 + ON32F] = 1.0
    put16(OFF_ON16, np.ones(8192, np.float32))
    return wsec


def build_in_maps(x, ps, conv1_w, conv1_b, conv2_w, conv2_b,
                  fc1_w, fc1_b, fc2_w, fc2_b):
    wsec = build_host_weights(ps, conv1_w, conv1_b, conv2_w, conv2_b,
                              fc1_w, fc1_b, fc2_w, fc2_b)
    x16 = np.asarray(x, np.float32).reshape(NCORES, BC * 784).astype(BFNP)
    in_maps = []
    for c in range(NCORES):
        blob = np.empty(NTOT, np.float32)
        blob[OFF_X:OFF_X + XF] = x16[c].view(np.float32)
        blob[OFF_L1:] = wsec
        in_maps.append({"blob": blob})
    return in_maps


def build_nc():
    nc = bacc.Bacc()
    blob_d = nc.dram_tensor("blob", [NTOT], F32R, kind="ExternalInput")
    out_d = nc.dram_tensor("out", [BC, 10], F32, kind="ExternalOutput")
    b32 = blob_d[:]
    b16 = blob_d.bitcast(BF16)[:]

    ctx = contextlib.ExitStack()
    with ctx:
        with TileContext(nc) as tc:
            with contextlib.ExitStack() as pctx:
                cpool = pctx.enter_context(tc.tile_pool(name="const", bufs=1))
                r1p = pctx.enter_context(tc.tile_pool(name="r1", bufs=2))
                p1p = pctx.enter_context(tc.tile_pool(name="p1", bufs=2))
                y1p = pctx.enter_context(tc.tile_pool(name="y1", bufs=2))
                c2rp = pctx.enter_context(tc.tile_pool(name="c2r", bufs=2))
                p2p = pctx.enter_context(tc.tile_pool(name="p2", bufs=2))
                t2p = pctx.enter_context(tc.tile_pool(name="t2", bufs=2))
                y2p = pctx.enter_context(tc.tile_pool(name="y2", bufs=2))
                y3p = pctx.enter_context(tc.tile_pool(name="y3", bufs=2))
                osbp = pctx.enter_context(tc.tile_pool(name="osb", bufs=2))
                e1p = pctx.enter_context(tc.tile_pool(name="e1", bufs=2))
                p1bp = pctx.enter_context(tc.tile_pool(name="p1b", bufs=2))
                p2bp = pctx.enter_context(tc.tile_pool(name="p2b", bufs=2))
                e2p = pctx.enter_context(tc.tile_pool(name="e2", bufs=2))
                ps1p = pctx.enter_context(tc.tile_pool(name="ps1", bufs=2, space="PSUM"))
                ps2p = pctx.enter_context(tc.tile_pool(name="ps2", bufs=2, space="PSUM"))
                ps3p = pctx.enter_context(tc.tile_pool(name="ps3", bufs=2, space="PSUM"))
                ps4p = pctx.enter_context(tc.tile_pool(name="ps4", bufs=2, space="PSUM"))
                # --- constants ---
                L1 = cpool.tile([41, 104], BF16)
                nc.sync.dma_start(
                    out=L1[:, :], in_=_ap(b16, 2 * OFF_L1, [[104, 41], [1, 104]]))
                L2 = cpool.tile([121, 570], BF16)
                nc.sync.dma_start(
                    out=L2[:, :], in_=_ap(b16, 2 * OFF_L2, [[570, 121], [1, 570]]))
                LF1 = cpool.tile([51, 8000], BF16)
                nc.sync.dma_start(
                    out=LF1[:, :], in_=_ap(b16, 2 * OFF_LF1, [[8000, 51], [1, 8000]]))
                LF2 = cpool.tile([126, 40], F32R)
                nc.sync.dma_start(
                    out=LF2[:, :], in_=_ap(b32, OFF_LF2, [[40, 126], [1, 40]]))

                y2_cur = None
                c2r_tiles = []
                for j in range(2):
                    t_ = c2rp.tile([121, CH * 48], BF16)
                    nc.sync.dma_start(
                        out=_ap(t_[:, :], 120 * _pitch(t_[:, :]),
                                [[_pitch(t_[:, :]), 1], [1, CH * 48]]),
                        in_=_ap(b16, 2 * OFF_ON16, [[0, 1], [1, CH * 48]]),
                    )
                    c2r_tiles.append(t_)
                for i in range(NCH):
                    q = i // QC
                    # ---- conv1 rhs: on-device im2col (30 DMAs + ones row) ----
                    R1 = r1p.tile([41, CH * 144], BF16)
                    pr = _pitch(R1[:, :])
                    nc.sync.dma_start(
                        out=_ap(R1[:, :], 40 * pr, [[pr, 1], [1, CH * 144]]),
                        in_=_ap(b16, 2 * OFF_ON16, [[0, 1], [1, CH * 144]]),
                    )
                    for dx in range(5):
                        for g in range(6):
                            eng = nc.sync if (dx * 6 + g) % 2 == 0 else nc.scalar
                            eng.dma_start(
                                out=_ap(R1[:, :], dx * 8 * pr + g * 24,
                                        [[pr, 8], [144, CH], [1, 24]]),
                                in_=_ap(b16, 2 * OFF_X + i * CH * 784 + g * 112 + dx,
                                        [[28, 8], [784, CH], [1, 24]]),
                            )
                    # ---- conv1 matmuls + evict + pool-x ----
                    P1 = p1p.tile([104, CH * 72], BF16)
                    pp1 = _pitch(P1[:, :])
                    for bs in range(CH // 2):
                        ps1 = ps1p.tile([104, 288], F32)
                        nc.tensor.matmul(
                            ps1[:, :], L1[:, :],
                            _ap(R1[:, :], bs * 288, [[pr, 41], [1, 288]]),
                            start=True, stop=True,
                        )
                        E1 = e1p.tile([104, 288], BF16)
                        pe1 = _pitch(E1[:, :])
                        nc.scalar.copy(out=E1[:, :], in_=ps1[:, :])
                        nc.vector.tensor_tensor(
                            out=_ap(P1[:, :], bs * 144,
                                    [[pp1, 104], [72, 2], [12, 6], [1, 12]]),
                            in0=_ap(E1[:, :], 0,
                                    [[pe1, 104], [144, 2], [24, 6], [2, 12]]),
                            in1=_ap(E1[:, :], 1,
                                    [[pe1, 104], [144, 2], [24, 6], [2, 12]]),
                            op=MAX,
                        )
                    # ---- conv1 pool-y + relu ----
                    P1B = p1bp.tile([40, CH * 72], BF16)
                    nc.sync.dma_start(out=P1B[:, :], in_=P1[64:104, :])
                    Y1 = y1p.tile([40, CH * 72], BF16)
                    nc.vector.tensor_tensor(
                        out=Y1[:, :], in0=P1[0:40, :], in1=P1B[:, :], op=MAX)
                    nc.vector.tensor_scalar_max(out=Y1[:, :], in0=Y1[:, :],
                                                scalar1=0.0)
                    # ---- shuffle Y1 -> C2R (6 DMAs) ----
                    C2R = c2r_tiles[i % 2]
                    pc = _pitch(C2R[:, :])
                    py1 = _pitch(Y1[:, :])
                    for u in range(2):
                        for v in range(3):
                            nc.sync.dma_start(
                                out=_ap(C2R[:, :], (2 * v + u) * 20 * pc,
                                        [[pc, 20], [48, CH], [1, 48]]),
                                in_=_ap(Y1[:, :], u * 20 * py1 + v * 12,
                                        [[py1, 20], [72, CH], [1, 48]]),
                            )
                    # ---- conv2: groups of 16 samples ----
                    P2 = p2p.tile([114, CH * 16], BF16)
                    pp2 = _pitch(P2[:, :])
                    for bg in range(CH // 16):
                        ps2 = ps2p.tile([114, 512], F32)
                        for dx in range(5):
                            nc.tensor.matmul(
                                ps2[:, :],
                                _ap(L2[:, :], dx * 114,
                                    [[_pitch(L2[:, :]), 121], [1, 114]]),
                                _ap(C2R[:, :], bg * 16 * 48 + dx,
                                    [[pc, 121], [48, 16], [12, 4], [1, 8]]),
                                start=(dx == 0), stop=(dx == 4),
                            )
                        E2 = e2p.tile([114, 512], BF16)
                        pe2 = _pitch(E2[:, :])
                        nc.scalar.copy(out=E2[:, :], in_=ps2[:, :])
                        nc.vector.tensor_tensor(
                            out=_ap(P2[:, :], bg * 256,
                                    [[pp2, 114], [16, 16], [4, 4], [1, 4]]),
                            in0=_ap(E2[:, :], 0,
                                    [[pe2, 114], [32, 16], [8, 4], [2, 4]]),
                            in1=_ap(E2[:, :], 1,
                                    [[pe2, 114], [32, 16], [8, 4], [2, 4]]),
                            op=MAX,
                        )
                    # ---- conv2 pool-y + bias/relu into Y2 ----
                    P2B = p2bp.tile([50, CH * 16], BF16)
                    nc.sync.dma_start(out=P2B[:, :], in_=P2[64:114, :])
                    T2 = t2p.tile([50, CH * 16], BF16)
                    nc.vector.tensor_tensor(
                        out=T2[:, :], in0=P2[0:50, :], in1=P2B[:, :], op=MAX)
                    if i % QC == 0:
                        y2_cur = y2p.tile([51, QC * CH * 16], BF16)
                        nc.sync.dma_start(
                            out=_ap(y2_cur[:, :], 50 * _pitch(y2_cur[:, :]),
                                    [[_pitch(y2_cur[:, :]), 1], [1, QC * CH * 16]]),
                            in_=_ap(b16, 2 * OFF_ON16, [[0, 1], [1, QC * CH * 16]]),
                        )
                    Y2 = y2_cur
                    nc.vector.tensor_scalar_max(
                        out=Y2[0:50, (i % QC) * CH * 16:(i % QC + 1) * CH * 16],
                        in0=T2[:, :], scalar1=0.0,
                    )
                    # ---- fc1 + fc2 per completed 256-sample group ----
                    if i % QC == QC - 1:
                        NB = QC * CH  # 256
                        py2 = _pitch(Y2[:, :])
                        Y3 = y3p.tile([126, 4 * NB], F32R)
                        nc.sync.dma_start(
                            out=_ap(Y3[:, :], 125 * _pitch(Y3[:, :]),
                                    [[_pitch(Y3[:, :]), 1], [1, 4 * NB]]),
                            in_=_ap(b32, OFF_ON32, [[0, 1], [1, 4 * NB]]),
                        )
                        for c in range(4):
                            ps3 = ps3p.tile([125, NB], F32)
                            for f in range(16):
                                nc.tensor.matmul(
                                    ps3[:, :],
                                    _ap(LF1[:, :], f * 500 + c * 125,
                                        [[_pitch(LF1[:, :]), 51], [1, 125]]),
                                    _ap(Y2[:, :], f, [[py2, 51], [16, NB]]),
                                    start=(f == 0), stop=(f == 15),
                                )
                            nc.vector.tensor_scalar_max(
                                out=Y3[0:125, c * NB:(c + 1) * NB],
                                in0=ps3[:, :], scalar1=0.0,
                            )
                        ps4 = ps4p.tile([10, NB], F32)
                        for c in range(4):
                            nc.tensor.matmul(
                                ps4[:, :],
                                _ap(LF2[:, :], c * 10,
                                    [[_pitch(LF2[:, :]), 126], [1, 10]]),
                                _ap(Y3[:, :], c * NB,
                                    [[_pitch(Y3[:, :]), 126], [1, NB]]),
                                start=(c == 0), stop=(c == 3),
                            )
                        OUT = osbp.tile([10, NB], F32)
                        nc.vector.tensor_copy(out=OUT[:, :], in_=ps4[:, :])
                        nc.sync.dma_start(
                            out=_ap(out_d[:], q * NB * 10, [[1, 10], [10, NB]]),
                            in_=_ap(OUT[:, :], 0, [[_pitch(OUT[:, :]), 10], [1, NB]]),
                        )
    return nc


_NC_CACHE = None


def kernel(x, ps, conv1_w, conv1_b, conv2_w, conv2_b, fc1_w, fc1_b, fc2_w, fc2_b):
    global _NC_CACHE
    from concourse import bass_utils

    if _NC_CACHE is None:
        _NC_CACHE = build_nc()
        _NC_CACHE.finalize()
    nc = _NC_CACHE

    in_maps = build_in_maps(x, ps, conv1_w, conv1_b, conv2_w, conv2_b,
                            fc1_w, fc1_b, fc2_w, fc2_b)
    res = bass_utils.run_bass_kernel_spmd(nc, in_maps, core_ids=list(range(NCORES)))
    out = np.concatenate([r["out"] for r in res.results], axis=0)
    return out.astype(np.float32)


# revision 12
# speedup vs baseline: 11.4852x; 2.1264x over previous
"""LeNet-style ClientNet (dense_cnn) on 8 Trainium2 NeuronCores.

Strategy (data-parallel, batch sharded 8x1024):
  host: ps-weighted average of the 16 client stacks (tiny einsum), weights
        pre-shaped into banded lhsT layouts for the PE; everything packed
        into ONE f32 blob per core (x + weights, bf16 where tolerable) so
        the axon-tunneled PJRT path pays a single per-array transfer.
  core: im2col for conv1 done on-device via strided DMAs from the raw x
        section of the blob (30 DMAs per 32-sample chunk), conv1 as one
        K=41 bf16 matmul per 2-sample block (banded rows + ones row for
        bias), relu+maxpool fused on DVE, conv2 as 5 dx-accumulated K=114
        bf16 matmuls, fc1 as 16 accumulated K=51 bf16 matmuls (one per
        spatial tap), fc2 K=126 x4 in f32r. psum always fp32.
"""

import contextlib
import sys

import numpy as np
import ml_dtypes

sys.path.insert(0, "/opt/trn_rl_repo")

import jax  # noqa: E402

# Persistent executable cache: the bass_exec HLO (which embeds the BIR) is
# byte-stable across calls, so cached executables skip the per-call walrus
# recompile (~0.3 s/call).
try:
    jax.config.update("jax_compilation_cache_dir", "/root/.jax_comp_cache")
    jax.config.update("jax_persistent_cache_min_compile_time_secs", 0.0)
    jax.config.update("jax_persistent_cache_min_entry_size_bytes", 0)
except Exception:
    pass

import concourse.bass as bass  # noqa: E402
import concourse.bacc as bacc  # noqa: E402
import concourse.mybir as mybir  # noqa: E402
from concourse.tile import TileContext  # noqa: E402

F32R = mybir.dt.float32r
F32 = mybir.dt.float32
BF16 = mybir.dt.bfloat16
FP8 = mybir.dt.float8e4
MAX = mybir.AluOpType.max
ADD = mybir.AluOpType.add
BFNP = ml_dtypes.bfloat16
F8NP = ml_dtypes.float8_e4m3

NCORES = 8
BC = 1024            # samples per core
CH = 32              # samples per chunk
NCH = BC // CH       # 32 chunks
QC = 8               # chunks per fc group (256 samples)
NQ = NCH // QC       # 4 fc groups

# ---- single-blob layout (units: f32 elements; bf16 sections hold 2/elem,
# fp8 sections hold 4/elem) ----
OFF_X = 0                       # x fp8 [1024,784]
XF = BC * 784 // 4              # 200704
OFF_ON8 = OFF_X + XF            # fp8 ones [4608]
ON8F = 4608 // 4                # 1152
OFF_L1 = OFF_ON8 + ON8F         # conv1 lhsT bf16 [41,104]
L1F = 41 * 104 // 2             # 2132
OFF_L2 = OFF_L1 + L1F           # conv2 lhsT bf16 [121,570]
L2F = 121 * 570 // 2            # 34485
OFF_LF1 = OFF_L2 + L2F          # fc1 lhsT bf16 [51,8000]
LF1F = 51 * 8000 // 2           # 204000
OFF_LF2 = OFF_LF1 + LF1F        # fc2 lhsT f32 [126,40]
LF2F = 126 * 40                 # 5040
OFF_ON32 = OFF_LF2 + LF2F       # f32 ones [4096]
ON32F = 4096
OFF_ON16 = OFF_ON32 + ON32F     # bf16 ones [8192]
ON16F = 8192 // 2
NTOT = OFF_ON16 + ON16F         # 655257


def _ap(t, off, dims):
    return bass.AP(tensor=t.tensor, offset=t.offset + off, ap=[list(d) for d in dims])


def _pitch(t):
    return t.ap[0][0]


def build_host_weights(ps, conv1_w, conv1_b, conv2_w, conv2_b,
                       fc1_w, fc1_b, fc2_w, fc2_b):
    ps = np.asarray(ps, np.float64)
    W1 = np.einsum("n,noihw->oihw", ps, np.asarray(conv1_w, np.float64))[:, 0]  # [20,5,5]
    b1 = ps @ np.asarray(conv1_b, np.float64)                                   # [20]
    W2 = np.einsum("n,noihw->oihw", ps, np.asarray(conv2_w, np.float64))        # [50,20,5,5]
    b2 = ps @ np.asarray(conv2_b, np.float64)                                   # [50]
    Wf1 = np.einsum("n,nof->of", ps, np.asarray(fc1_w, np.float64))             # [500,800]
    bf1 = ps @ np.asarray(fc1_b, np.float64)                                    # [500]
    Wf2 = np.einsum("n,nof->of", ps, np.asarray(fc2_w, np.float64))             # [10,500]
    bf2 = ps @ np.asarray(fc2_b, np.float64)                                    # [10]

    # conv1 lhsT [41, 104]: k = dx*8 + rr (rows 0..39), row 40 = bias ones-row.
    # m = e*64 + u*20 + o ; out row y = 4G + 2u + e ; input row 4G + rr,
    # dy = rr - (2u + e) in 0..4.
    L1 = np.zeros((41, 104), np.float32)
    for dx in range(5):
        for rr in range(8):
            for e in range(2):
                for u in range(2):
                    for o in range(20):
                        dy = rr - (2 * u + e)
                        if 0 <= dy <= 4:
                            L1[dx * 8 + rr, e * 64 + u * 20 + o] = W1[o, dy, dx]
    for e in range(2):
        for u in range(2):
            for o in range(20):
                L1[40, e * 64 + u * 20 + o] = b1[o]

    # conv2 lhsT [121, 570]: k = rr*20 + c, m(dx) = dx*114 + e*64 + o.
    # out row y' = 2gg + e ; pooled input row 2gg + rr ; dy = rr - e.
    L2 = np.zeros((121, 570), np.float32)
    for dx in range(5):
        for c in range(20):
            for rr in range(6):
                for e in range(2):
                    dy = rr - e
                    if 0 <= dy <= 4:
                        L2[rr * 20 + c, dx * 114 + e * 64:dx * 114 + e * 64 + 50] = \
                            W2[:, c, dy, dx]
    for e in range(2):
        L2[120, e * 64:e * 64 + 50] = b2

    # fc1 lhsT [51, 16*500]: tap f = gg*4 + xp; torch feature id = o*16 + f.
    LF1 = np.zeros((51, 8000), np.float32)
    for gg in range(4):
        for xp in range(4):
            f = gg * 4 + xp
            for o in range(50):
                LF1[o, f * 500:(f + 1) * 500] = Wf1[:, o * 16 + f]
    LF1[50, 0:500] = bf1

    # fc2 lhsT [126, 40]
    LF2 = np.zeros((126, 40), np.float32)
    for c in range(4):
        LF2[0:125, c * 10:(c + 1) * 10] = Wf2[:, c * 125:(c + 1) * 125].T
    LF2[125, 0:10] = bf2

    # ---- pack weight sections of the blob (f32 view) ----
    wsec = np.zeros(NTOT - OFF_ON8, np.float32)

    def put16(off_f32, arr):
        v = np.ascontiguousarray(arr.astype(BFNP)).reshape(-1).view(np.float32)
        wsec[off_f32 - OFF_ON8:off_f32 - OFF_ON8 + v.size] = v

    v8 = np.ones(4608, F8NP).reshape(-1).view(np.float32)
    wsec[0:ON8F] = v8
    put16(OFF_L1, L1)
    put16(OFF_L2, L2)
    put16(OFF_LF1, LF1)
    wsec[OFF_LF2 - OFF_ON8:OFF_LF2 - OFF_ON8 + LF2F] = LF2.reshape(-1)
    wsec[OFF_ON32 - OFF_ON8:OFF_ON32 - OFF_ON8 + ON32F] = 1.0
    put16(OFF_ON16, np.ones(8192, np.float32))
    return wsec


def build_in_maps(x, ps, conv1_w, conv1_b, conv2_w, conv2_b,
                  fc1_w, fc1_b, fc2_w, fc2_b):
    wsec = build_host_weights(ps, conv1_w, conv1_b, conv2_w, conv2_b,
                              fc1_w, fc1_b, fc2_w, fc2_b)
    x8 = np.asarray(x, np.float32).reshape(NCORES, BC * 784).astype(F8NP)
    in_maps = []
    for c in range(NCORES):
        blob = np.empty(NTOT, np.float32)
        blob[OFF_X:OFF_X + XF] = x8[c].view(np.float32)
        blob[OFF_ON8:] = wsec
        in_maps.append({"blob": blob})
    return in_maps


def build_nc():
    nc = bacc.Bacc()
    blob_d = nc.dram_tensor("blob", [NTOT], F32R, kind="ExternalInput")
    out_d = nc.dram_tensor("out", [BC, 10], F32, kind="ExternalOutput")
    b32 = blob_d[:]
    b16 = blob_d.bitcast(BF16)[:]
    b8 = blob_d.bitcast(FP8)[:]

    ctx = contextlib.ExitStack()
    with ctx:
        with TileContext(nc) as tc:
            with contextlib.ExitStack() as pctx:
                cpool = pctx.enter_context(tc.tile_pool(name="const", bufs=1))
                r8p = pctx.enter_context(tc.tile_pool(name="r8", bufs=2))
                r1p = pctx.enter_context(tc.tile_pool(name="r1", bufs=2))
                p1p = pctx.enter_context(tc.tile_pool(name="p1", bufs=2))
                y1p = pctx.enter_context(tc.tile_pool(name="y1", bufs=2))
                c2rp = pctx.enter_context(tc.tile_pool(name="c2r", bufs=2))
                p2p = pctx.enter_context(tc.tile_pool(name="p2", bufs=2))
                t2p = pctx.enter_context(tc.tile_pool(name="t2", bufs=2))
                y2p = pctx.enter_context(tc.tile_pool(name="y2", bufs=2))
                y3p = pctx.enter_context(tc.tile_pool(name="y3", bufs=2))
                osbp = pctx.enter_context(tc.tile_pool(name="osb", bufs=2))
                e1p = pctx.enter_context(tc.tile_pool(name="e1", bufs=2))
                p1bp = pctx.enter_context(tc.tile_pool(name="p1b", bufs=2))
                p2bp = pctx.enter_context(tc.tile_pool(name="p2b", bufs=2))
                e2p = pctx.enter_context(tc.tile_pool(name="e2", bufs=2))
                ps1p = pctx.enter_context(tc.tile_pool(name="ps1", bufs=2, space="PSUM"))
                ps2p = pctx.enter_context(tc.tile_pool(name="ps2", bufs=2, space="PSUM"))
                ps3p = pctx.enter_context(tc.tile_pool(name="ps3", bufs=2, space="PSUM"))
                ps4p = pctx.enter_context(tc.tile_pool(name="ps4", bufs=2, space="PSUM"))
                # --- constants ---
                L1 = cpool.tile([41, 104], BF16)
                nc.sync.dma_start(
                    out=L1[:, :], in_=_ap(b16, 2 * OFF_L1, [[104, 41], [1, 104]]))
                L2 = cpool.tile([121, 570], BF16)
                nc.sync.dma_start(
                    out=L2[:, :], in_=_ap(b16, 2 * OFF_L2, [[570, 121], [1, 570]]))
                LF1 = cpool.tile([51, 8000], BF16)
                nc.sync.dma_start(
                    out=LF1[:, :], in_=_ap(b16, 2 * OFF_LF1, [[8000, 51], [1, 8000]]))
                LF2 = cpool.tile([126, 40], F32R)
                nc.sync.dma_start(
                    out=LF2[:, :], in_=_ap(b32, OFF_LF2, [[40, 126], [1, 40]]))

                y2_cur = None
                c2r_tiles = []
                for j in range(2):
                    t_ = c2rp.tile([121, CH * 48], BF16)
                    nc.sync.dma_start(
                        out=_ap(t_[:, :], 120 * _pitch(t_[:, :]),
                                [[_pitch(t_[:, :]), 1], [1, CH * 48]]),
                        in_=_ap(b16, 2 * OFF_ON16, [[0, 1], [1, CH * 48]]),
                    )
                    c2r_tiles.append(t_)
                for i in range(NCH):
                    q = i // QC
                    # ---- conv1 rhs: on-device im2col in fp8 (30 DMAs + ones
                    # row), then one gpsimd upconvert to bf16 ----
                    R8 = r8p.tile([41, CH * 144], FP8)
                    p8 = _pitch(R8[:, :])
                    nc.sync.dma_start(
                        out=_ap(R8[:, :], 40 * p8, [[p8, 1], [1, CH * 144]]),
                        in_=_ap(b8, 4 * OFF_ON8, [[0, 1], [1, CH * 144]]),
                    )
                    for dx in range(5):
                        for g in range(6):
                            eng = nc.sync if (dx * 6 + g) % 2 == 0 else nc.scalar
                            eng.dma_start(
                                out=_ap(R8[:, :], dx * 8 * p8 + g * 24,
                                        [[p8, 8], [144, CH], [1, 24]]),
                                in_=_ap(b8, 4 * OFF_X + i * CH * 784 + g * 112 + dx,
                                        [[28, 8], [784, CH], [1, 24]]),
                            )
                    R1 = r1p.tile([41, CH * 144], BF16)
                    pr = _pitch(R1[:, :])
                    nc.vector.tensor_copy(out=R1[:, :], in_=R8[:, :])
                    # ---- conv1 matmuls + evict + pool-x ----
                    P1 = p1p.tile([104, CH * 72], BF16)
                    pp1 = _pitch(P1[:, :])
                    for bs in range(CH // 2):
                        ps1 = ps1p.tile([104, 288], F32)
                        nc.tensor.matmul(
                            ps1[:, :], L1[:, :],
                            _ap(R1[:, :], bs * 288, [[pr, 41], [1, 288]]),
                            start=True, stop=True,
                        )
                        E1 = e1p.tile([104, 288], BF16)
                        pe1 = _pitch(E1[:, :])
                        nc.scalar.copy(out=E1[:, :], in_=ps1[:, :])
                        nc.vector.tensor_tensor(
                            out=_ap(P1[:, :], bs * 144,
                                    [[pp1, 104], [72, 2], [12, 6], [1, 12]]),
                            in0=_ap(E1[:, :], 0,
                                    [[pe1, 104], [144, 2], [24, 6], [2, 12]]),
                            in1=_ap(E1[:, :], 1,
                                    [[pe1, 104], [144, 2], [24, 6], [2, 12]]),
                            op=MAX,
                        )
                    # ---- conv1 pool-y + relu ----
                    P1B = p1bp.tile([40, CH * 72], BF16)
                    nc.sync.dma_start(out=P1B[:, :], in_=P1[64:104, :])
                    Y1 = y1p.tile([40, CH * 72], BF16)
                    nc.vector.tensor_tensor(
                        out=Y1[:, :], in0=P1[0:40, :], in1=P1B[:, :], op=MAX)
                    nc.vector.tensor_scalar_max(out=Y1[:, :], in0=Y1[:, :],
                                                scalar1=0.0)
                    # ---- shuffle Y1 -> C2R (6 DMAs) ----
                    C2R = c2r_tiles[i % 2]
                    pc = _pitch(C2R[:, :])
                    py1 = _pitch(Y1[:, :])
                    for u in range(2):
                        for v in range(3):
                            nc.sync.dma_start(
                                out=_ap(C2R[:, :], (2 * v + u) * 20 * pc,
                                        [[pc, 20], [48, CH], [1, 48]]),
                                in_=_ap(Y1[:, :], u * 20 * py1 + v * 12,
                                        [[py1, 20], [72, CH], [1, 48]]),
                            )
                    # ---- conv2: groups of 16 samples ----
                    P2 = p2p.tile([114, CH * 16], BF16)
                    pp2 = _pitch(P2[:, :])
                    for bg in range(CH // 16):
                        ps2 = ps2p.tile([114, 512], F32)
                        for dx in range(5):
                            nc.tensor.matmul(
                                ps2[:, :],
                                _ap(L2[:, :], dx * 114,
                                    [[_pitch(L2[:, :]), 121], [1, 114]]),
                                _ap(C2R[:, :], bg * 16 * 48 + dx,
                                    [[pc, 121], [48, 16], [12, 4], [1, 8]]),
                                start=(dx == 0), stop=(dx == 4),
                            )
                        E2 = e2p.tile([114, 512], BF16)
                        pe2 = _pitch(E2[:, :])
                        nc.scalar.copy(out=E2[:, :], in_=ps2[:, :])
                        nc.vector.tensor_tensor(
                            out=_ap(P2[:, :], bg * 256,
                                    [[pp2, 114], [16, 16], [4, 4], [1, 4]]),
                            in0=_ap(E2[:, :], 0,
                                    [[pe2, 114], [32, 16], [8, 4], [2, 4]]),
                            in1=_ap(E2[:, :], 1,
                                    [[pe2, 114], [32, 16], [8, 4], [2, 4]]),
                            op=MAX,
                        )
                    # ---- conv2 pool-y + bias/relu into Y2 ----
                    P2B = p2bp.tile([50, CH * 16], BF16)
                    nc.sync.dma_start(out=P2B[:, :], in_=P2[64:114, :])
                    T2 = t2p.tile([50, CH * 16], BF16)
                    nc.vector.tensor_tensor(
                        out=T2[:, :], in0=P2[0:50, :], in1=P2B[:, :], op=MAX)
                    if i % QC == 0:
                        y2_cur = y2p.tile([51, QC * CH * 16], BF16)
                        nc.sync.dma_start(
                            out=_ap(y2_cur[:, :], 50 * _pitch(y2_cur[:, :]),
                                    [[_pitch(y2_cur[:, :]), 1], [1, QC * CH * 16]]),
                            in_=_ap(b16, 2 * OFF_ON16, [[0, 1], [1, QC * CH * 16]]),
                        )
                    Y2 = y2_cur
                    nc.vector.tensor_scalar_max(
                        out=Y2[0:50, (i % QC) * CH * 16:(i % QC + 1) * CH * 16],
                        in0=T2[:, :], scalar1=0.0,
                    )
                    # ---- fc1 + fc2 per completed 256-sample group ----
                    if i % QC == QC - 1:
                        # All Y2 writes of this group must land before fc1
                        # reads them: sim-correct scheduling raced on HW here.
                        tc.strict_bb_all_engine_barrier()
                        NB = QC * CH  # 256
                        py2 = _pitch(Y2[:, :])
                        Y3 = y3p.tile([126, 4 * NB], F32R)
                        nc.sync.dma_start(
                            out=_ap(Y3[:, :], 125 * _pitch(Y3[:, :]),
                                    [[_pitch(Y3[:, :]), 1], [1, 4 * NB]]),
                            in_=_ap(b32, OFF_ON32, [[0, 1], [1, 4 * NB]]),
                        )
                        for c in range(4):
                            ps3 = ps3p.tile([125, NB], F32)
                            for f in range(16):
                                nc.tensor.matmul(
                                    ps3[:, :],
                                    _ap(LF1[:, :], f * 500 + c * 125,
                                        [[_pitch(LF1[:, :]), 51], [1, 125]]),
                                    _ap(Y2[:, :], f, [[py2, 51], [16, NB]]),
                                    start=(f == 0), stop=(f == 15),
                                )
                            nc.vector.tensor_scalar_max(
                                out=Y3[0:125, c * NB:(c + 1) * NB],
                                in0=ps3[:, :], scalar1=0.0,
                            )
                        ps4 = ps4p.tile([10, NB], F32)
                        for c in range(4):
                            nc.tensor.matmul(
                                ps4[:, :],
                                _ap(LF2[:, :], c * 10,
                                    [[_pitch(LF2[:, :]), 126], [1, 10]]),
                                _ap(Y3[:, :], c * NB,
                                    [[_pitch(Y3[:, :]), 126], [1, NB]]),
                                start=(c == 0), stop=(c == 3),
                            )
                        OUT = osbp.tile([10, NB], F32)
                        nc.vector.tensor_copy(out=OUT[:, :], in_=ps4[:, :])
                        nc.sync.dma_start(
                            out=_ap(out_d[:], q * NB * 10, [[1, 10], [10, NB]]),
                            in_=_ap(OUT[:, :], 0, [[_pitch(OUT[:, :]), 10], [1, NB]]),
                        )
    return nc


_NC_CACHE = None


def kernel(x, ps, conv1_w, conv1_b, conv2_w, conv2_b, fc1_w, fc1_b, fc2_w, fc2_b):
    global _NC_CACHE
    from concourse import bass_utils

    if _NC_CACHE is None:
        _NC_CACHE = build_nc()
        _NC_CACHE.finalize()
    nc = _NC_CACHE

    in_maps = build_in_maps(x, ps, conv1_w, conv1_b, conv2_w, conv2_b,
                            fc1_w, fc1_b, fc2_w, fc2_b)
    res = bass_utils.run_bass_kernel_spmd(nc, in_maps, core_ids=list(range(NCORES)))
    out = np.concatenate([r["out"] for r in res.results], axis=0)
    return out.astype(np.float32)


# revision 15
# speedup vs baseline: 13.2239x; 1.1514x over previous
"""LeNet-style ClientNet (dense_cnn) on 8 Trainium2 NeuronCores.

Strategy (data-parallel, batch sharded 8x1024):
  host: ps-weighted average of the 16 client stacks (tiny einsum), weights
        pre-shaped into banded lhsT layouts for the PE; everything packed
        into ONE f32 blob per core (x + weights, bf16 where tolerable) so
        the axon-tunneled PJRT path pays a single per-array transfer.
  core: im2col for conv1 done on-device via strided DMAs from the raw x
        section of the blob (30 DMAs per 32-sample chunk), conv1 as one
        K=41 bf16 matmul per 2-sample block (banded rows + ones row for
        bias), relu+maxpool fused on DVE, conv2 as 5 dx-accumulated K=114
        bf16 matmuls, fc1 as 16 accumulated K=51 bf16 matmuls (one per
        spatial tap), fc2 K=126 x4 in f32r. psum always fp32.
"""

import contextlib
import sys

import numpy as np
import ml_dtypes

sys.path.insert(0, "/opt/trn_rl_repo")

import jax  # noqa: E402

# Persistent executable cache: the bass_exec HLO (which embeds the BIR) is
# byte-stable across calls, so cached executables skip the per-call walrus
# recompile (~0.3 s/call).
try:
    jax.config.update("jax_compilation_cache_dir", "/root/.jax_comp_cache")
    jax.config.update("jax_persistent_cache_min_compile_time_secs", 0.0)
    jax.config.update("jax_persistent_cache_min_entry_size_bytes", 0)
except Exception:
    pass

import concourse.bass as bass  # noqa: E402
import concourse.bacc as bacc  # noqa: E402
import concourse.mybir as mybir  # noqa: E402
from concourse.tile import TileContext  # noqa: E402

F32R = mybir.dt.float32r
F32 = mybir.dt.float32
F16 = mybir.dt.float16
BF16 = mybir.dt.bfloat16
FP8 = mybir.dt.float8e4
MAX = mybir.AluOpType.max
ADD = mybir.AluOpType.add
BFNP = ml_dtypes.bfloat16
F8NP = ml_dtypes.float8_e4m3

NCORES = 8
BC = 1024            # samples per core
CH = 32              # samples per chunk
NCH = BC // CH       # 32 chunks
QC = 8               # chunks per fc group (256 samples)
NQ = NCH // QC       # 4 fc groups

# ---- single-blob layout (units: f32 elements; bf16 sections hold 2/elem,
# fp8 sections hold 4/elem) ----
OFF_X = 0                       # x fp8 [1024,784]
XF = BC * 784 // 4              # 200704
OFF_ON8 = OFF_X + XF            # fp8 ones [4608]
ON8F = 4608 // 4                # 1152
OFF_L1 = OFF_ON8 + ON8F         # conv1 lhsT bf16 [41,104]
L1F = 41 * 104 // 2             # 2132
OFF_L2 = OFF_L1 + L1F           # conv2 lhsT bf16 [121,570]
L2F = 121 * 570 // 2            # 34485
OFF_LF1 = OFF_L2 + L2F          # fc1 lhsT bf16 [51,8000]
LF1F = 51 * 8000 // 2           # 204000
OFF_LF2 = OFF_LF1 + LF1F        # fc2 lhsT f32 [126,40]
LF2F = 126 * 40                 # 5040
OFF_ON32 = OFF_LF2 + LF2F       # f32 ones [4096]
ON32F = 4096
OFF_ON16 = OFF_ON32 + ON32F     # bf16 ones [8192]
ON16F = 8192 // 2
NTOT = OFF_ON16 + ON16F         # 655257


def _ap(t, off, dims):
    return bass.AP(tensor=t.tensor, offset=t.offset + off, ap=[list(d) for d in dims])


def _pitch(t):
    return t.ap[0][0]


def build_host_weights(ps, conv1_w, conv1_b, conv2_w, conv2_b,
                       fc1_w, fc1_b, fc2_w, fc2_b):
    ps = np.asarray(ps, np.float64)
    W1 = np.einsum("n,noihw->oihw", ps, np.asarray(conv1_w, np.float64))[:, 0]  # [20,5,5]
    b1 = ps @ np.asarray(conv1_b, np.float64)                                   # [20]
    W2 = np.einsum("n,noihw->oihw", ps, np.asarray(conv2_w, np.float64))        # [50,20,5,5]
    b2 = ps @ np.asarray(conv2_b, np.float64)                                   # [50]
    Wf1 = np.einsum("n,nof->of", ps, np.asarray(fc1_w, np.float64))             # [500,800]
    bf1 = ps @ np.asarray(fc1_b, np.float64)                                    # [500]
    Wf2 = np.einsum("n,nof->of", ps, np.asarray(fc2_w, np.float64))             # [10,500]
    bf2 = ps @ np.asarray(fc2_b, np.float64)                                    # [10]

    # conv1 lhsT [41, 104]: k = dx*8 + rr (rows 0..39), row 40 = bias ones-row.
    # m = e*64 + u*20 + o ; out row y = 4G + 2u + e ; input row 4G + rr,
    # dy = rr - (2u + e) in 0..4.
    L1 = np.zeros((41, 104), np.float32)
    for dx in range(5):
        for rr in range(8):
            for e in range(2):
                for u in range(2):
                    for o in range(20):
                        dy = rr - (2 * u + e)
                        if 0 <= dy <= 4:
                            L1[dx * 8 + rr, e * 64 + u * 20 + o] = W1[o, dy, dx]
    for e in range(2):
        for u in range(2):
            for o in range(20):
                L1[40, e * 64 + u * 20 + o] = b1[o]

    # conv2 lhsT [121, 570]: k = rr*20 + c, m(dx) = dx*114 + e*64 + o.
    # out row y' = 2gg + e ; pooled input row 2gg + rr ; dy = rr - e.
    L2 = np.zeros((121, 570), np.float32)
    for dx in range(5):
        for c in range(20):
            for rr in range(6):
                for e in range(2):
                    dy = rr - e
                    if 0 <= dy <= 4:
                        L2[rr * 20 + c, dx * 114 + e * 64:dx * 114 + e * 64 + 50] = \
                            W2[:, c, dy, dx]
    for e in range(2):
        L2[120, e * 64:e * 64 + 50] = b2

    # fc1 lhsT [51, 16*500]: tap f = gg*4 + xp; torch feature id = o*16 + f.
    LF1 = np.zeros((51, 8000), np.float32)
    for gg in range(4):
        for xp in range(4):
            f = gg * 4 + xp
            for o in range(50):
                LF1[o, f * 500:(f + 1) * 500] = Wf1[:, o * 16 + f]
    LF1[50, 0:500] = bf1

    # fc2 lhsT [126, 40]
    LF2 = np.zeros((126, 40), np.float32)
    for c in range(4):
        LF2[0:125, c * 10:(c + 1) * 10] = Wf2[:, c * 125:(c + 1) * 125].T
    LF2[125, 0:10] = bf2

    # ---- pack weight sections of the blob (f32 view) ----
    wsec = np.zeros(NTOT - OFF_ON8, np.float32)

    def put16(off_f32, arr):
        v = np.ascontiguousarray(arr.astype(BFNP)).reshape(-1).view(np.float32)
        wsec[off_f32 - OFF_ON8:off_f32 - OFF_ON8 + v.size] = v

    v8 = np.ones(4608, F8NP).reshape(-1).view(np.float32)
    wsec[0:ON8F] = v8
    put16(OFF_L1, L1)
    put16(OFF_L2, L2)
    put16(OFF_LF1, LF1)
    wsec[OFF_LF2 - OFF_ON8:OFF_LF2 - OFF_ON8 + LF2F] = LF2.reshape(-1)
    wsec[OFF_ON32 - OFF_ON8:OFF_ON32 - OFF_ON8 + ON32F] = 1.0
    put16(OFF_ON16, np.ones(8192, np.float32))
    return wsec


def build_in_maps(x, ps, conv1_w, conv1_b, conv2_w, conv2_b,
                  fc1_w, fc1_b, fc2_w, fc2_b):
    wsec = build_host_weights(ps, conv1_w, conv1_b, conv2_w, conv2_b,
                              fc1_w, fc1_b, fc2_w, fc2_b)
    x8 = np.asarray(x, np.float32).reshape(NCORES, BC * 784).astype(F8NP)
    in_maps = []
    for c in range(NCORES):
        blob = np.empty(NTOT, np.float32)
        blob[OFF_X:OFF_X + XF] = x8[c].view(np.float32)
        blob[OFF_ON8:] = wsec
        in_maps.append({"blob": blob})
    return in_maps


def build_nc():
    nc = bacc.Bacc()
    blob_d = nc.dram_tensor("blob", [NTOT], F32R, kind="ExternalInput")
    out_d = nc.dram_tensor("out", [BC, 10], F16, kind="ExternalOutput")
    b32 = blob_d[:]
    b16 = blob_d.bitcast(BF16)[:]
    b8 = blob_d.bitcast(FP8)[:]

    ctx = contextlib.ExitStack()
    with ctx:
        with TileContext(nc) as tc:
            with contextlib.ExitStack() as pctx:
                cpool = pctx.enter_context(tc.tile_pool(name="const", bufs=1))
                r8p = pctx.enter_context(tc.tile_pool(name="r8", bufs=2))
                r1p = pctx.enter_context(tc.tile_pool(name="r1", bufs=2))
                p1p = pctx.enter_context(tc.tile_pool(name="p1", bufs=2))
                y1p = pctx.enter_context(tc.tile_pool(name="y1", bufs=2))
                c2rp = pctx.enter_context(tc.tile_pool(name="c2r", bufs=2))
                p2p = pctx.enter_context(tc.tile_pool(name="p2", bufs=2))
                t2p = pctx.enter_context(tc.tile_pool(name="t2", bufs=2))
                y2p = pctx.enter_context(tc.tile_pool(name="y2", bufs=2))
                y3p = pctx.enter_context(tc.tile_pool(name="y3", bufs=2))
                osbp = pctx.enter_context(tc.tile_pool(name="osb", bufs=2))
                p1ep = pctx.enter_context(tc.tile_pool(name="p1e", bufs=2))
                p1bp = pctx.enter_context(tc.tile_pool(name="p1b", bufs=2))
                p2bp = pctx.enter_context(tc.tile_pool(name="p2b", bufs=2))
                e2p = pctx.enter_context(tc.tile_pool(name="e2", bufs=2))
                ps1p = pctx.enter_context(tc.tile_pool(name="ps1", bufs=2, space="PSUM"))
                ps2p = pctx.enter_context(tc.tile_pool(name="ps2", bufs=2, space="PSUM"))
                ps3p = pctx.enter_context(tc.tile_pool(name="ps3", bufs=2, space="PSUM"))
                ps4p = pctx.enter_context(tc.tile_pool(name="ps4", bufs=2, space="PSUM"))
                # --- constants ---
                L1 = cpool.tile([41, 104], BF16)
                nc.sync.dma_start(
                    out=L1[:, :], in_=_ap(b16, 2 * OFF_L1, [[104, 41], [1, 104]]))
                L2 = cpool.tile([121, 570], BF16)
                nc.sync.dma_start(
                    out=L2[:, :], in_=_ap(b16, 2 * OFF_L2, [[570, 121], [1, 570]]))
                LF1 = cpool.tile([51, 8000], BF16)
                nc.sync.dma_start(
                    out=LF1[:, :], in_=_ap(b16, 2 * OFF_LF1, [[8000, 51], [1, 8000]]))
                LF2 = cpool.tile([126, 40], F32R)
                nc.sync.dma_start(
                    out=LF2[:, :], in_=_ap(b32, OFF_LF2, [[40, 126], [1, 40]]))

                y2_cur = None
                c2r_tiles = []
                for j in range(2):
                    t_ = c2rp.tile([121, CH * 48], BF16)
                    nc.sync.dma_start(
                        out=_ap(t_[:, :], 120 * _pitch(t_[:, :]),
                                [[_pitch(t_[:, :]), 1], [1, CH * 48]]),
                        in_=_ap(b16, 2 * OFF_ON16, [[0, 1], [1, CH * 48]]),
                    )
                    c2r_tiles.append(t_)
                for i in range(NCH):
                    q = i // QC
                    # ---- conv1 rhs: on-device im2col in fp8 (30 DMAs + ones
                    # row), then one gpsimd upconvert to bf16 ----
                    R8 = r8p.tile([41, CH * 144], FP8)
                    p8 = _pitch(R8[:, :])
                    nc.sync.dma_start(
                        out=_ap(R8[:, :], 40 * p8, [[p8, 1], [1, CH * 144]]),
                        in_=_ap(b8, 4 * OFF_ON8, [[0, 1], [1, CH * 144]]),
                    )
                    for dx in range(5):
                        for g in range(6):
                            eng = nc.sync if (dx * 6 + g) % 2 == 0 else nc.scalar
                            eng.dma_start(
                                out=_ap(R8[:, :], dx * 8 * p8 + g * 24,
                                        [[p8, 8], [144, CH], [1, 24]]),
                                in_=_ap(b8, 4 * OFF_X + i * CH * 784 + g * 112 + dx,
                                        [[28, 8], [784, CH], [1, 24]]),
                            )
                    R1 = r1p.tile([41, CH * 144], BF16)
                    pr = _pitch(R1[:, :])
                    nc.vector.tensor_copy(out=R1[:, :], in_=R8[:, :])
                    # ---- conv1: 9 N=512 matmuls, relu fused into the psum evict,
                    # one pool-x DVE op for the whole chunk ----
                    P1E = p1ep.tile([104, CH * 144], BF16)
                    ppe = _pitch(P1E[:, :])
                    for bs in range(CH * 144 // 512):
                        ps1 = ps1p.tile([104, 512], F32)
                        nc.tensor.matmul(
                            ps1[:, :], L1[:, :],
                            _ap(R1[:, :], bs * 512, [[pr, 41], [1, 512]]),
                            start=True, stop=True,
                        )
                        nc.scalar.activation(
                            out=P1E[:, bs * 512:(bs + 1) * 512], in_=ps1[:, :],
                            func=mybir.ActivationFunctionType.Relu)
                    P1 = p1p.tile([104, CH * 72], BF16)
                    pp1 = _pitch(P1[:, :])
                    nc.vector.tensor_tensor(
                        out=_ap(P1[:, :], 0,
                                [[pp1, 104], [72, CH], [12, 6], [1, 12]]),
                        in0=_ap(P1E[:, :], 0,
                                [[ppe, 104], [144, CH], [24, 6], [2, 12]]),
                        in1=_ap(P1E[:, :], 1,
                                [[ppe, 104], [144, CH], [24, 6], [2, 12]]),
                        op=MAX,
                    )
                    # ---- conv1 pool-y (inputs already relu'd) ----
                    P1B = p1bp.tile([40, CH * 72], BF16)
                    nc.sync.dma_start(out=P1B[:, :], in_=P1[64:104, :])
                    Y1 = y1p.tile([40, CH * 72], BF16)
                    nc.vector.tensor_tensor(
                        out=Y1[:, :], in0=P1[0:40, :], in1=P1B[:, :], op=MAX)
                    # ---- shuffle Y1 -> C2R (6 DMAs) ----
                    C2R = c2r_tiles[i % 2]
                    pc = _pitch(C2R[:, :])
                    py1 = _pitch(Y1[:, :])
                    for u in range(2):
                        for v in range(3):
                            nc.sync.dma_start(
                                out=_ap(C2R[:, :], (2 * v + u) * 20 * pc,
                                        [[pc, 20], [48, CH], [1, 48]]),
                                in_=_ap(Y1[:, :], u * 20 * py1 + v * 12,
                                        [[py1, 20], [72, CH], [1, 48]]),
                            )
                    # ---- conv2: groups of 16 samples ----
                    P2 = p2p.tile([114, CH * 16], BF16)
                    pp2 = _pitch(P2[:, :])
                    for bg in range(CH // 16):
                        ps2 = ps2p.tile([114, 512], F32)
                        for dx in range(5):
                            nc.tensor.matmul(
                                ps2[:, :],
                                _ap(L2[:, :], dx * 114,
                                    [[_pitch(L2[:, :]), 121], [1, 114]]),
                                _ap(C2R[:, :], bg * 16 * 48 + dx,
                                    [[pc, 121], [48, 16], [12, 4], [1, 8]]),
                                start=(dx == 0), stop=(dx == 4),
                            )
                        E2 = e2p.tile([114, 512], BF16)
                        pe2 = _pitch(E2[:, :])
                        nc.scalar.copy(out=E2[:, :], in_=ps2[:, :])
                        nc.vector.tensor_tensor(
                            out=_ap(P2[:, :], bg * 256,
                                    [[pp2, 114], [16, 16], [4, 4], [1, 4]]),
                            in0=_ap(E2[:, :], 0,
                                    [[pe2, 114], [32, 16], [8, 4], [2, 4]]),
                            in1=_ap(E2[:, :], 1,
                                    [[pe2, 114], [32, 16], [8, 4], [2, 4]]),
                            op=MAX,
                        )
                    # ---- conv2 pool-y + bias/relu into Y2 ----
                    P2B = p2bp.tile([50, CH * 16], BF16)
                    nc.sync.dma_start(out=P2B[:, :], in_=P2[64:114, :])
                    T2 = t2p.tile([50, CH * 16], BF16)
                    nc.vector.tensor_tensor(
                        out=T2[:, :], in0=P2[0:50, :], in1=P2B[:, :], op=MAX)
                    if i % QC == 0:
                        y2_cur = y2p.tile([51, QC * CH * 16], BF16)
                        nc.sync.dma_start(
                            out=_ap(y2_cur[:, :], 50 * _pitch(y2_cur[:, :]),
                                    [[_pitch(y2_cur[:, :]), 1], [1, QC * CH * 16]]),
                            in_=_ap(b16, 2 * OFF_ON16, [[0, 1], [1, QC * CH * 16]]),
                        )
                    Y2 = y2_cur
                    nc.vector.tensor_scalar_max(
                        out=Y2[0:50, (i % QC) * CH * 16:(i % QC + 1) * CH * 16],
                        in0=T2[:, :], scalar1=0.0,
                    )
                    # ---- fc1 + fc2 per completed 256-sample group ----
                    if i % QC == QC - 1:
                        # All Y2 writes of this group must land before fc1
                        # reads them: sim-correct scheduling raced on HW here.
                        tc.strict_bb_all_engine_barrier()
                        NB = QC * CH  # 256
                        py2 = _pitch(Y2[:, :])
                        Y3 = y3p.tile([126, 4 * NB], F32R)
                        nc.sync.dma_start(
                            out=_ap(Y3[:, :], 125 * _pitch(Y3[:, :]),
                                    [[_pitch(Y3[:, :]), 1], [1, 4 * NB]]),
                            in_=_ap(b32, OFF_ON32, [[0, 1], [1, 4 * NB]]),
                        )
                        for c in range(4):
                            ps3 = ps3p.tile([125, NB], F32)
                            for f in range(16):
                                nc.tensor.matmul(
                                    ps3[:, :],
                                    _ap(LF1[:, :], f * 500 + c * 125,
                                        [[_pitch(LF1[:, :]), 51], [1, 125]]),
                                    _ap(Y2[:, :], f, [[py2, 51], [16, NB]]),
                                    start=(f == 0), stop=(f == 15),
                                )
                            nc.vector.tensor_scalar_max(
                                out=Y3[0:125, c * NB:(c + 1) * NB],
                                in0=ps3[:, :], scalar1=0.0,
                            )
                        ps4 = ps4p.tile([10, NB], F32)
                        for c in range(4):
                            nc.tensor.matmul(
                                ps4[:, :],
                                _ap(LF2[:, :], c * 10,
                                    [[_pitch(LF2[:, :]), 126], [1, 10]]),
                                _ap(Y3[:, :], c * NB,
                                    [[_pitch(Y3[:, :]), 126], [1, NB]]),
                                start=(c == 0), stop=(c == 3),
                            )
                        OUT = osbp.tile([10, NB], F16)
                        nc.vector.tensor_copy(out=OUT[:, :], in_=ps4[:, :])
                        nc.sync.dma_start(
                            out=_ap(out_d[:], q * NB * 10, [[1, 10], [10, NB]]),
                            in_=_ap(OUT[:, :], 0, [[_pitch(OUT[:, :]), 10], [1, NB]]),
                        )
    return nc


_NC_CACHE = None


def kernel(x, ps, conv1_w, conv1_b, conv2_w, conv2_b, fc1_w, fc1_b, fc2_w, fc2_b):
    global _NC_CACHE
    from concourse import bass_utils

    if _NC_CACHE is None:
        _NC_CACHE = build_nc()
        _NC_CACHE.finalize()
    nc = _NC_CACHE

    in_maps = build_in_maps(x, ps, conv1_w, conv1_b, conv2_w, conv2_b,
                            fc1_w, fc1_b, fc2_w, fc2_b)
    res = bass_utils.run_bass_kernel_spmd(nc, in_maps, core_ids=list(range(NCORES)))
    out = np.concatenate([r["out"] for r in res.results], axis=0)
    return out.astype(np.float32)
